# revision 5
# baseline (speedup 1.0000x reference)
"""Trainium2 Bass kernel for the JoraLayer problem.

out = x @ W_eff.T + bias, where
  W_eff = rowrot(colrot(W, pairs_R, theta_R), pairs_L, theta_L) rescaled so
  row n has norm base_row_norms[n]*exp(ecd_log_mag[n]) (up to EPS).

Strategy (8 cores, tensor-parallel over output rows n):
  - pairs_L / pairs_R are disjoint permutations (each index appears exactly
    once), so rotations can be made stride-regular by reindexing:
      * contraction dim m reordered by m_perm = [pairs_R[:,0], pairs_R[:,1]];
        device row k pairs with k+2048 => m-tile t pairs with m-tile t+16 at
        the same partition -> column rotation is 2 full-tile DVE combines.
      * output rows per core ordered [pairs_c[:,0], pairs_c[:,1]] (256 pairs
        per core) => free position l pairs with l+256 -> row rotation is a
        free-dim-half DVE combine.
  - W shard is stored transposed: Wt[k, l] = W[n_order[l], m_perm[k]].
  - x replicated, transposed+permuted on host: xT = x.T[m_perm]  [4096, 8192].
  - Matmuls run in float32r (fp32 data, PE truncates to fp22): full PE rate,
    ~1e-4 relative error.
  - Row-norm rescale folded into the PSUM->SBUF output evacuation.
  - Output computed transposed outT[n, t] per core; host scatters back.
"""

import os
import sys
from contextlib import ExitStack

import numpy as np

sys.path.insert(0, "/opt/trn_rl_repo")

N = 4096
M = 4096
T = 8192
S_L = 2048
S_R = 2048
EPS = 1e-6

NCORES = 8
NC_ROWS = N // NCORES        # 512 output rows per core
PPC = S_L // NCORES          # 256 row-pairs per core
P = 128                      # partitions
MT = M // P                  # 32 m-tiles
HALF_MT = MT // 2            # 16 (m-tile t pairs with t+16)
NT = NC_ROWS // P            # 4 n-subtiles of 128
F = 512                      # token-slab width / matmul free dim
ST = T // F                  # 16 slabs
XBUFS = 36                   # rolling x-tile buffers

_NC_CACHE = {}


def _build_nc():
    import concourse.bacc as bacc
    import concourse.tile as tile
    from concourse import mybir

    f32 = mybir.dt.float32
    f32r = mybir.dt.float32r
    Alu = mybir.AluOpType
    Act = mybir.ActivationFunctionType

    nc = bacc.Bacc("TRN2", target_bir_lowering=False, debug=False)

    xT_h = nc.dram_tensor("xT", [M, T], f32r, kind="ExternalInput")
    Wt_h = nc.dram_tensor("Wt", [M, NC_ROWS], f32, kind="ExternalInput")
    cosR_h = nc.dram_tensor("cosR", [S_R], f32, kind="ExternalInput")
    sinR_h = nc.dram_tensor("sinR", [S_R], f32, kind="ExternalInput")
    cosL_h = nc.dram_tensor("cosL", [PPC], f32, kind="ExternalInput")
    sinL_h = nc.dram_tensor("sinL", [PPC], f32, kind="ExternalInput")
    sfac_h = nc.dram_tensor("sfac", [NC_ROWS], f32, kind="ExternalInput")
    bias_h = nc.dram_tensor("biasv", [NC_ROWS], f32, kind="ExternalInput")
    outT_h = nc.dram_tensor("outT", [NC_ROWS, T], f32, kind="ExternalOutput")

    with ExitStack() as ctx:
        tc = ctx.enter_context(tile.TileContext(nc))
        smalls = ctx.enter_context(tc.tile_pool(name="smalls", bufs=1))
        dramp = ctx.enter_context(tc.tile_pool(name="dramp", bufs=1, space="DRAM"))
        weffp = ctx.enter_context(tc.tile_pool(name="weffp", bufs=MT))
        winp = ctx.enter_context(tc.tile_pool(name="winp", bufs=2))
        t512 = ctx.enter_context(tc.tile_pool(name="t512", bufs=2))
        t256 = ctx.enter_context(tc.tile_pool(name="t256", bufs=2))
        sqp = ctx.enter_context(tc.tile_pool(name="sqp", bufs=2))
        xp = ctx.enter_context(tc.tile_pool(name="xp", bufs=XBUFS))
        outp = ctx.enter_context(tc.tile_pool(name="outp", bufs=4))
        psmain = ctx.enter_context(tc.tile_pool(name="psmain", bufs=6, space="PSUM"))
        psaux = ctx.enter_context(tc.tile_pool(name="psaux", bufs=2, space="PSUM"))

        # ---- constants / small vectors -------------------------------------
        cosR_sb = smalls.tile([P, HALF_MT], f32)  # [p, t] = cosR[128 t + p]
        nc.sync.dma_start(out=cosR_sb, in_=cosR_h[:].rearrange("(t p) -> p t", p=P))
        sinR_sb = smalls.tile([P, HALF_MT], f32)
        nc.sync.dma_start(out=sinR_sb, in_=sinR_h[:].rearrange("(t p) -> p t", p=P))

        cosL_row = smalls.tile([1, PPC], f32)
        nc.sync.dma_start(out=cosL_row, in_=cosL_h[:].rearrange("(a f) -> a f", a=1))
        sinL_row = smalls.tile([1, PPC], f32)
        nc.sync.dma_start(out=sinL_row, in_=sinL_h[:].rearrange("(a f) -> a f", a=1))
        sfac_row = smalls.tile([1, NC_ROWS], f32)
        nc.sync.dma_start(out=sfac_row, in_=sfac_h[:].rearrange("(a f) -> a f", a=1))
        bias_pt = smalls.tile([P, NT], f32)  # [p, n] = bias[n*128 + p]
        nc.sync.dma_start(out=bias_pt, in_=bias_h[:].rearrange("(n p) -> p n", p=P))

        ones1 = smalls.tile([1, P], f32)
        nc.vector.memset(ones1, 1.0)
        ones128f = smalls.tile([P, 1], f32)
        nc.vector.memset(ones128f, 1.0)
        ones128 = smalls.tile([P, 1], f32r)
        nc.vector.tensor_copy(out=ones128, in_=ones128f)

        # broadcast cosL/sinL along partitions via outer product ones^T @ row
        ps_bc = psaux.tile([P, PPC], f32, name="ps_bc", tag="psbc")
        nc.tensor.matmul(out=ps_bc, lhsT=ones1, rhs=cosL_row, start=True, stop=True)
        cL_bc = smalls.tile([P, PPC], f32)
        nc.vector.tensor_copy(out=cL_bc, in_=ps_bc)
        ps_bc2 = psaux.tile([P, PPC], f32, name="ps_bc2", tag="psbc")
        nc.tensor.matmul(out=ps_bc2, lhsT=ones1, rhs=sinL_row, start=True, stop=True)
        sL_bc = smalls.tile([P, PPC], f32)
        nc.vector.tensor_copy(out=sL_bc, in_=ps_bc2)

        # ---- W preparation: column rotation + row rotation -----------------
        weff = {}
        for i in range(MT):
            weff[i] = weffp.tile([P, NC_ROWS], f32r, name=f"weff{i}", tag="weff")

        for t in range(HALF_MT):
            wa = winp.tile([P, NC_ROWS], f32, name=f"wa{t}", tag="wa")
            nc.sync.dma_start(out=wa, in_=Wt_h[P * t : P * (t + 1), :])
            wb = winp.tile([P, NC_ROWS], f32, name=f"wb{t}", tag="wb")
            nc.sync.dma_start(out=wb, in_=Wt_h[P * (t + HALF_MT) : P * (t + HALF_MT + 1), :])
            cR = cosR_sb[:, t : t + 1]
            sR = sinR_sb[:, t : t + 1]
            # column rotation (per-partition scalars):
            #   ca = cR*wa - sR*wb ; cb = sR*wa + cR*wb
            ca = t512.tile([P, NC_ROWS], f32, name=f"ca{t}", tag="ca")
            nc.vector.tensor_scalar_mul(ca, wa, cR)
            u1 = t512.tile([P, NC_ROWS], f32, name=f"u1{t}", tag="u1")
            nc.vector.tensor_scalar_mul(u1, wb, sR)
            nc.vector.tensor_sub(ca, ca, u1)
            cb = t512.tile([P, NC_ROWS], f32, name=f"cb{t}", tag="cb")
            nc.vector.tensor_scalar_mul(cb, wb, cR)
            u2 = t512.tile([P, NC_ROWS], f32, name=f"u2{t}", tag="u2")
            nc.vector.tensor_scalar_mul(u2, wa, sR)
            nc.vector.tensor_add(cb, cb, u2)
            # row rotation (free-dim halves), writes final weff tiles:
            #   out[:, :256] = cL*src_L - sL*src_R
            #   out[:, 256:] = sL*src_L + cL*src_R
            for src, wi in ((ca, t), (cb, t + HALF_MT)):
                dst = weff[wi]
                sl = src[:, 0:PPC]
                sr = src[:, PPC : 2 * PPC]
                p1 = t256.tile([P, PPC], f32, name=f"p1_{wi}", tag="p1")
                nc.vector.tensor_mul(p1, sl, cL_bc)
                p2 = t256.tile([P, PPC], f32, name=f"p2_{wi}", tag="p2")
                nc.vector.tensor_mul(p2, sr, sL_bc)
                nc.vector.tensor_sub(dst[:, 0:PPC], p1, p2)
                p3 = t256.tile([P, PPC], f32, name=f"p3_{wi}", tag="p3")
                nc.vector.tensor_mul(p3, sl, sL_bc)
                p4 = t256.tile([P, PPC], f32, name=f"p4_{wi}", tag="p4")
                nc.vector.tensor_mul(p4, sr, cL_bc)
                nc.vector.tensor_add(dst[:, PPC : 2 * PPC], p3, p4)

        # ---- row norms of W_eff (partition-sum via ones matmul) ------------
        norm_ps = psaux.tile([1, NC_ROWS], f32, name="norm_ps", tag="psbc")
        for i in range(MT):
            sq = sqp.tile([P, NC_ROWS], f32r, name=f"sq{i}", tag="sq")
            nc.vector.tensor_mul(sq, weff[i], weff[i])
            nc.tensor.matmul(
                out=norm_ps,
                lhsT=ones128,
                rhs=sq,
                start=(i == 0),
                stop=(i == MT - 1),
            )

        # scale = sfac / sqrt(norm2 + EPS), relayout [1,512] -> [128,4]
        eps_tile = smalls.tile([1, 1], f32)
        nc.vector.memset(eps_tile, EPS)
        sc_sqrt = smalls.tile([1, NC_ROWS], f32)
        nc.scalar.activation(out=sc_sqrt, in_=norm_ps, func=Act.Sqrt, bias=eps_tile)
        sc_rec = smalls.tile([1, NC_ROWS], f32)
        nc.vector.reciprocal(out=sc_rec, in_=sc_sqrt)
        scale_row = smalls.tile([1, NC_ROWS], f32)
        nc.vector.tensor_mul(scale_row, sfac_row, sc_rec)
        scratch = dramp.tile([NC_ROWS], f32, name="scratch")
        nc.sync.dma_start(out=scratch[:].rearrange("(a f) -> a f", a=1), in_=scale_row)
        scale_pt = smalls.tile([P, NT], f32)
        nc.sync.dma_start(out=scale_pt, in_=scratch[:].rearrange("(n p) -> p n", p=P))

        # ---- main matmul: outT[n, t] = sum_m weff[m, n] * xT[m, t] ---------
        for s in range(ST):
            ps = []
            for n in range(NT):
                pst = psmain.tile([P, F], f32, name=f"ps_{s}_{n}", tag="ps")
                ps.append(pst)
            for m in range(MT):
                xt = xp.tile([P, F], f32r, name=f"xt_{s}_{m}", tag="xt")
                nc.sync.dma_start(
                    out=xt, in_=xT_h[P * m : P * (m + 1), F * s : F * (s + 1)]
                )
                for n in range(NT):
                    nc.tensor.matmul(
                        out=ps[n],
                        lhsT=weff[m][:, P * n : P * (n + 1)],
                        rhs=xt,
                        start=(m == 0),
                        stop=(m == MT - 1),
                    )
            for n in range(NT):
                ob = outp.tile([P, F], f32, name=f"ob_{s}_{n}", tag="ob")
                nc.vector.tensor_scalar(
                    ob,
                    ps[n],
                    scale_pt[:, n : n + 1],
                    bias_pt[:, n : n + 1],
                    Alu.mult,
                    Alu.add,
                )
                nc.sync.dma_start(
                    out=outT_h[P * n : P * (n + 1), F * s : F * (s + 1)], in_=ob
                )

    nc.compile()
    return nc


def _get_nc():
    if "nc" not in _NC_CACHE:
        _NC_CACHE["nc"] = _build_nc()
    return _NC_CACHE["nc"]


def _numpy_reference(x, W, bias, theta_L, theta_R, ecd_log_mag, base_row_norms,
                     pairs_L, pairs_R):
    """Fallback: exact reference math in numpy (used only if pairs are not
    disjoint permutations, which setup_inputs never produces)."""
    Wm = W.astype(np.float64).copy()
    i, j = pairs_R[:, 0], pairs_R[:, 1]
    c = np.cos(theta_R.astype(np.float64))
    s = np.sin(theta_R.astype(np.float64))
    ci, cj = Wm[:, i].copy(), Wm[:, j].copy()
    Wm[:, i] = c[None, :] * ci - s[None, :] * cj
    Wm[:, j] = s[None, :] * ci + c[None, :] * cj
    i, j = pairs_L[:, 0], pairs_L[:, 1]
    c = np.cos(theta_L.astype(np.float64))
    s = np.sin(theta_L.astype(np.float64))
    ri, rj = Wm[i].copy(), Wm[j].copy()
    Wm[i] = c[:, None] * ri - s[:, None] * rj
    Wm[j] = s[:, None] * ri + c[:, None] * rj
    norms = np.sqrt((Wm * Wm).sum(axis=1) + EPS)
    scale = base_row_norms.astype(np.float64) * np.exp(
        ecd_log_mag.astype(np.float64)) / norms
    Wm *= scale[:, None]
    return (x.astype(np.float64) @ Wm.T + bias.astype(np.float64)).astype(np.float32)


LAST_RESULTS = None


def kernel(x, W, bias, theta_L, theta_R, ecd_log_mag, base_row_norms,
           pairs_L, pairs_R):
    from concourse.bass_utils import run_bass_kernel_spmd

    x = np.ascontiguousarray(np.asarray(x, dtype=np.float32))
    W = np.ascontiguousarray(np.asarray(W, dtype=np.float32))
    bias = np.asarray(bias, dtype=np.float32)
    theta_L = np.asarray(theta_L, dtype=np.float32)
    theta_R = np.asarray(theta_R, dtype=np.float32)
    ecd_log_mag = np.asarray(ecd_log_mag, dtype=np.float32)
    base_row_norms = np.asarray(base_row_norms, dtype=np.float32)
    pairs_L = np.asarray(pairs_L, dtype=np.int32)
    pairs_R = np.asarray(pairs_R, dtype=np.int32)

    if (np.sort(pairs_L.reshape(-1)) != np.arange(N, dtype=np.int32)).any() or (
        np.sort(pairs_R.reshape(-1)) != np.arange(M, dtype=np.int32)
    ).any():
        return _numpy_reference(x, W, bias, theta_L, theta_R, ecd_log_mag,
                                base_row_norms, pairs_L, pairs_R)

    m_perm = np.concatenate([pairs_R[:, 0], pairs_R[:, 1]])  # [4096]
    xT = np.ascontiguousarray(x.T[m_perm])                   # [M, T]
    cosR = np.cos(theta_R.astype(np.float64)).astype(np.float32)
    sinR = np.sin(theta_R.astype(np.float64)).astype(np.float32)

    in_maps = []
    orders = []
    for cidx in range(NCORES):
        pc = pairs_L[cidx * PPC : (cidx + 1) * PPC]
        n_order = np.concatenate([pc[:, 0], pc[:, 1]])       # [512]
        orders.append(n_order)
        Wt = np.ascontiguousarray(W[n_order][:, m_perm].T)   # [M, 512]
        thL = theta_L[cidx * PPC : (cidx + 1) * PPC].astype(np.float64)
        sfac = (base_row_norms[n_order].astype(np.float64)
                * np.exp(ecd_log_mag[n_order].astype(np.float64)))
        in_maps.append(
            dict(
                xT=xT,
                Wt=Wt,
                cosR=cosR,
                sinR=sinR,
                cosL=np.cos(thL).astype(np.float32),
                sinL=np.sin(thL).astype(np.float32),
                sfac=sfac.astype(np.float32),
                biasv=bias[n_order].astype(np.float32),
            )
        )

    nc = _get_nc()
    run_kwargs = {}
    td = os.environ.get("BASS_TMPDIR")
    if td:
        os.makedirs(td, exist_ok=True)
        run_kwargs["tmpdir"] = td
    res = run_bass_kernel_spmd(nc, in_maps, core_ids=list(range(NCORES)), **run_kwargs)
    global LAST_RESULTS
    LAST_RESULTS = res

    out = np.empty((T, N), dtype=np.float32)
    for cidx in range(NCORES):
        out[:, orders[cidx]] = res.results[cidx]["outT"].T
    return out


# revision 6
# speedup vs baseline: 1.0268x; 1.0268x over previous
"""Trainium2 Bass kernel for the JoraLayer problem.

out = x @ W_eff.T + bias, where
  W_eff = rowrot(colrot(W, pairs_R, theta_R), pairs_L, theta_L) rescaled so
  row n has norm base_row_norms[n]*exp(ecd_log_mag[n]) (up to EPS).

Strategy (8 cores, tensor-parallel over output rows n):
  - pairs_L / pairs_R are disjoint permutations (each index appears exactly
    once), so rotations can be made stride-regular by reindexing:
      * contraction dim m reordered by m_perm = [pairs_R[:,0], pairs_R[:,1]];
        device row k pairs with k+2048 => m-tile t pairs with m-tile t+16 at
        the same partition -> column rotation is 2 full-tile DVE combines.
      * output rows per core ordered [pairs_c[:,0], pairs_c[:,1]] (256 pairs
        per core) => free position l pairs with l+256 -> row rotation is a
        free-dim-half DVE combine.
  - W shard is stored transposed: Wt[k, l] = W[n_order[l], m_perm[k]].
  - x replicated, transposed+permuted on host: xT = x.T[m_perm]  [4096, 8192].
  - Matmuls run in float32r (fp32 data, PE truncates to fp22): full PE rate,
    ~1e-4 relative error.
  - Row-norm rescale folded into the PSUM->SBUF output evacuation.
  - Output computed transposed outT[n, t] per core; host scatters back.
"""

import os
import sys
from contextlib import ExitStack

import numpy as np

sys.path.insert(0, "/opt/trn_rl_repo")

N = 4096
M = 4096
T = 8192
S_L = 2048
S_R = 2048
EPS = 1e-6

NCORES = 8
NC_ROWS = N // NCORES        # 512 output rows per core
PPC = S_L // NCORES          # 256 row-pairs per core
P = 128                      # partitions
MT = M // P                  # 32 m-tiles
HALF_MT = MT // 2            # 16 (m-tile t pairs with t+16)
NT = NC_ROWS // P            # 4 n-subtiles of 128
F = 512                      # token-slab width / matmul free dim
ST = T // F                  # 16 slabs
XBUFS = 10                   # rolling packed x-tile buffers (1 MiB each)
# consume m-tiles in W-prep pair-completion order so slab-0 matmuls can start
# while later pairs are still being rotated
MS_ORDER = [t + h * HALF_MT for t in range(HALF_MT) for h in (0, 1)]

_NC_CACHE = {}


def _build_nc():
    import concourse.bacc as bacc
    import concourse.tile as tile
    from concourse import mybir

    f32 = mybir.dt.float32
    f32r = mybir.dt.float32r
    Alu = mybir.AluOpType
    Act = mybir.ActivationFunctionType

    nc = bacc.Bacc("TRN2", target_bir_lowering=False, debug=False)

    xT_h = nc.dram_tensor("xT", [ST, MT // 4, P, 4, F], f32r, kind="ExternalInput")
    Wt_h = nc.dram_tensor("Wt", [M, NC_ROWS], f32, kind="ExternalInput")
    cosR_h = nc.dram_tensor("cosR", [S_R], f32, kind="ExternalInput")
    sinR_h = nc.dram_tensor("sinR", [S_R], f32, kind="ExternalInput")
    cosL_h = nc.dram_tensor("cosL", [PPC], f32, kind="ExternalInput")
    sinL_h = nc.dram_tensor("sinL", [PPC], f32, kind="ExternalInput")
    sfac_h = nc.dram_tensor("sfac", [NC_ROWS], f32, kind="ExternalInput")
    bias_h = nc.dram_tensor("biasv", [NC_ROWS], f32, kind="ExternalInput")
    outT_h = nc.dram_tensor("outT", [ST, NT, P, F], f32, kind="ExternalOutput")

    with ExitStack() as ctx:
        tc = ctx.enter_context(tile.TileContext(nc))
        smalls = ctx.enter_context(tc.tile_pool(name="smalls", bufs=1))
        dramp = ctx.enter_context(tc.tile_pool(name="dramp", bufs=1, space="DRAM"))
        weffp = ctx.enter_context(tc.tile_pool(name="weffp", bufs=MT))
        winp = ctx.enter_context(tc.tile_pool(name="winp", bufs=2))
        t512 = ctx.enter_context(tc.tile_pool(name="t512", bufs=2))
        t256 = ctx.enter_context(tc.tile_pool(name="t256", bufs=2))
        sqp = ctx.enter_context(tc.tile_pool(name="sqp", bufs=2))
        xp = ctx.enter_context(tc.tile_pool(name="xp", bufs=XBUFS))
        outp = ctx.enter_context(tc.tile_pool(name="outp", bufs=4))
        psmain = ctx.enter_context(tc.tile_pool(name="psmain", bufs=6, space="PSUM"))
        psaux = ctx.enter_context(tc.tile_pool(name="psaux", bufs=2, space="PSUM"))

        # ---- constants / small vectors -------------------------------------
        cosR_sb = smalls.tile([P, HALF_MT], f32)  # [p, t] = cosR[128 t + p]
        nc.sync.dma_start(out=cosR_sb, in_=cosR_h[:].rearrange("(t p) -> p t", p=P))
        sinR_sb = smalls.tile([P, HALF_MT], f32)
        nc.sync.dma_start(out=sinR_sb, in_=sinR_h[:].rearrange("(t p) -> p t", p=P))

        cosL_row = smalls.tile([1, PPC], f32)
        nc.sync.dma_start(out=cosL_row, in_=cosL_h[:].rearrange("(a f) -> a f", a=1))
        sinL_row = smalls.tile([1, PPC], f32)
        nc.sync.dma_start(out=sinL_row, in_=sinL_h[:].rearrange("(a f) -> a f", a=1))
        sfac_row = smalls.tile([1, NC_ROWS], f32)
        nc.sync.dma_start(out=sfac_row, in_=sfac_h[:].rearrange("(a f) -> a f", a=1))
        bias_pt = smalls.tile([P, NT], f32)  # [p, n] = bias[n*128 + p]
        nc.sync.dma_start(out=bias_pt, in_=bias_h[:].rearrange("(n p) -> p n", p=P))

        ones1 = smalls.tile([1, P], f32)
        nc.vector.memset(ones1, 1.0)
        ones128f = smalls.tile([P, 1], f32)
        nc.vector.memset(ones128f, 1.0)
        ones128 = smalls.tile([P, 1], f32r)
        nc.vector.tensor_copy(out=ones128, in_=ones128f)

        # broadcast cosL/sinL along partitions via outer product ones^T @ row
        ps_bc = psaux.tile([P, PPC], f32, name="ps_bc", tag="psbc")
        nc.tensor.matmul(out=ps_bc, lhsT=ones1, rhs=cosL_row, start=True, stop=True)
        cL_bc = smalls.tile([P, PPC], f32)
        nc.vector.tensor_copy(out=cL_bc, in_=ps_bc)
        ps_bc2 = psaux.tile([P, PPC], f32, name="ps_bc2", tag="psbc")
        nc.tensor.matmul(out=ps_bc2, lhsT=ones1, rhs=sinL_row, start=True, stop=True)
        sL_bc = smalls.tile([P, PPC], f32)
        nc.vector.tensor_copy(out=sL_bc, in_=ps_bc2)

        # ---- W preparation: column rotation + row rotation -----------------
        weff = {}
        for i in range(MT):
            weff[i] = weffp.tile([P, NC_ROWS], f32r, name=f"weff{i}", tag="weff")

        for t in range(HALF_MT):
            wa = winp.tile([P, NC_ROWS], f32, name=f"wa{t}", tag="wa")
            nc.sync.dma_start(out=wa, in_=Wt_h[P * t : P * (t + 1), :])
            wb = winp.tile([P, NC_ROWS], f32, name=f"wb{t}", tag="wb")
            nc.sync.dma_start(out=wb, in_=Wt_h[P * (t + HALF_MT) : P * (t + HALF_MT + 1), :])
            cR = cosR_sb[:, t : t + 1]
            sR = sinR_sb[:, t : t + 1]
            # column rotation (per-partition scalars):
            #   ca = cR*wa - sR*wb ; cb = sR*wa + cR*wb
            ca = t512.tile([P, NC_ROWS], f32, name=f"ca{t}", tag="ca")
            nc.vector.tensor_scalar_mul(ca, wa, cR)
            u1 = t512.tile([P, NC_ROWS], f32, name=f"u1{t}", tag="u1")
            nc.vector.tensor_scalar_mul(u1, wb, sR)
            nc.vector.tensor_sub(ca, ca, u1)
            cb = t512.tile([P, NC_ROWS], f32, name=f"cb{t}", tag="cb")
            nc.vector.tensor_scalar_mul(cb, wb, cR)
            u2 = t512.tile([P, NC_ROWS], f32, name=f"u2{t}", tag="u2")
            nc.vector.tensor_scalar_mul(u2, wa, sR)
            nc.vector.tensor_add(cb, cb, u2)
            # row rotation (free-dim halves), writes final weff tiles:
            #   out[:, :256] = cL*src_L - sL*src_R
            #   out[:, 256:] = sL*src_L + cL*src_R
            for src, wi in ((ca, t), (cb, t + HALF_MT)):
                dst = weff[wi]
                sl = src[:, 0:PPC]
                sr = src[:, PPC : 2 * PPC]
                p1 = t256.tile([P, PPC], f32, name=f"p1_{wi}", tag="p1")
                nc.vector.tensor_mul(p1, sl, cL_bc)
                p2 = t256.tile([P, PPC], f32, name=f"p2_{wi}", tag="p2")
                nc.vector.tensor_mul(p2, sr, sL_bc)
                nc.vector.tensor_sub(dst[:, 0:PPC], p1, p2)
                p3 = t256.tile([P, PPC], f32, name=f"p3_{wi}", tag="p3")
                nc.vector.tensor_mul(p3, sl, sL_bc)
                p4 = t256.tile([P, PPC], f32, name=f"p4_{wi}", tag="p4")
                nc.vector.tensor_mul(p4, sr, cL_bc)
                nc.vector.tensor_add(dst[:, PPC : 2 * PPC], p3, p4)

        # ---- row norms of W_eff (partition-sum via ones matmul) ------------
        norm_ps = psaux.tile([1, NC_ROWS], f32, name="norm_ps", tag="psbc")
        for k, i in enumerate(MS_ORDER):
            sq = sqp.tile([P, NC_ROWS], f32r, name=f"sq{i}", tag="sq")
            nc.vector.tensor_mul(sq, weff[i], weff[i])
            nc.tensor.matmul(
                out=norm_ps,
                lhsT=ones128,
                rhs=sq,
                start=(k == 0),
                stop=(k == MT - 1),
            )

        # scale = sfac / sqrt(norm2 + EPS), relayout [1,512] -> [128,4]
        eps_tile = smalls.tile([1, 1], f32)
        nc.vector.memset(eps_tile, EPS)
        sc_sqrt = smalls.tile([1, NC_ROWS], f32)
        nc.scalar.activation(out=sc_sqrt, in_=norm_ps, func=Act.Sqrt, bias=eps_tile)
        sc_rec = smalls.tile([1, NC_ROWS], f32)
        nc.vector.reciprocal(out=sc_rec, in_=sc_sqrt)
        scale_row = smalls.tile([1, NC_ROWS], f32)
        nc.vector.tensor_mul(scale_row, sfac_row, sc_rec)
        scratch = dramp.tile([NC_ROWS], f32, name="scratch")
        nc.sync.dma_start(out=scratch[:].rearrange("(a f) -> a f", a=1), in_=scale_row)
        scale_pt = smalls.tile([P, NT], f32)
        nc.sync.dma_start(out=scale_pt, in_=scratch[:].rearrange("(n p) -> p n", p=P))

        # ---- main matmul: outT[n, t] = sum_m weff[m, n] * xT[m, t] ---------
        NG = MT // 4  # 8 x-DMA groups per slab, 4 m-tiles each
        for s in range(ST):
            ps = []
            for n in range(NT):
                pst = psmain.tile([P, F], f32, name=f"ps_{s}_{n}", tag="ps")
                ps.append(pst)
            for g in range(NG):
                xt4 = xp.tile([P, 4, F], f32r, name=f"xt_{s}_{g}", tag="xt")
                nc.sync.dma_start(out=xt4, in_=xT_h[s, g])
                for mi in range(4):
                    m = MS_ORDER[4 * g + mi]
                    for n in range(NT):
                        nc.tensor.matmul(
                            out=ps[n],
                            lhsT=weff[m][:, P * n : P * (n + 1)],
                            rhs=xt4[:, mi, :],
                            start=(g == 0 and mi == 0),
                            stop=(g == NG - 1 and mi == 3),
                        )
            for n in range(NT):
                ob = outp.tile([P, F], f32, name=f"ob_{s}_{n}", tag="ob")
                nc.vector.tensor_scalar(
                    ob,
                    ps[n],
                    scale_pt[:, n : n + 1],
                    bias_pt[:, n : n + 1],
                    Alu.mult,
                    Alu.add,
                )
                nc.sync.dma_start(out=outT_h[s, n], in_=ob)

    nc.compile()
    return nc


def _get_nc():
    if "nc" not in _NC_CACHE:
        _NC_CACHE["nc"] = _build_nc()
    return _NC_CACHE["nc"]


def _numpy_reference(x, W, bias, theta_L, theta_R, ecd_log_mag, base_row_norms,
                     pairs_L, pairs_R):
    """Fallback: exact reference math in numpy (used only if pairs are not
    disjoint permutations, which setup_inputs never produces)."""
    Wm = W.astype(np.float64).copy()
    i, j = pairs_R[:, 0], pairs_R[:, 1]
    c = np.cos(theta_R.astype(np.float64))
    s = np.sin(theta_R.astype(np.float64))
    ci, cj = Wm[:, i].copy(), Wm[:, j].copy()
    Wm[:, i] = c[None, :] * ci - s[None, :] * cj
    Wm[:, j] = s[None, :] * ci + c[None, :] * cj
    i, j = pairs_L[:, 0], pairs_L[:, 1]
    c = np.cos(theta_L.astype(np.float64))
    s = np.sin(theta_L.astype(np.float64))
    ri, rj = Wm[i].copy(), Wm[j].copy()
    Wm[i] = c[:, None] * ri - s[:, None] * rj
    Wm[j] = s[:, None] * ri + c[:, None] * rj
    norms = np.sqrt((Wm * Wm).sum(axis=1) + EPS)
    scale = base_row_norms.astype(np.float64) * np.exp(
        ecd_log_mag.astype(np.float64)) / norms
    Wm *= scale[:, None]
    return (x.astype(np.float64) @ Wm.T + bias.astype(np.float64)).astype(np.float32)


LAST_RESULTS = None


def kernel(x, W, bias, theta_L, theta_R, ecd_log_mag, base_row_norms,
           pairs_L, pairs_R):
    from concourse.bass_utils import run_bass_kernel_spmd

    x = np.ascontiguousarray(np.asarray(x, dtype=np.float32))
    W = np.ascontiguousarray(np.asarray(W, dtype=np.float32))
    bias = np.asarray(bias, dtype=np.float32)
    theta_L = np.asarray(theta_L, dtype=np.float32)
    theta_R = np.asarray(theta_R, dtype=np.float32)
    ecd_log_mag = np.asarray(ecd_log_mag, dtype=np.float32)
    base_row_norms = np.asarray(base_row_norms, dtype=np.float32)
    pairs_L = np.asarray(pairs_L, dtype=np.int32)
    pairs_R = np.asarray(pairs_R, dtype=np.int32)

    if (np.sort(pairs_L.reshape(-1)) != np.arange(N, dtype=np.int32)).any() or (
        np.sort(pairs_R.reshape(-1)) != np.arange(M, dtype=np.int32)
    ).any():
        return _numpy_reference(x, W, bias, theta_L, theta_R, ecd_log_mag,
                                base_row_norms, pairs_L, pairs_R)

    m_perm = np.concatenate([pairs_R[:, 0], pairs_R[:, 1]])  # [4096]
    # rows of x.T in device m order, with m-tiles permuted to MS_ORDER
    rows = m_perm.reshape(MT, P)[np.asarray(MS_ORDER)].reshape(-1)
    xg = x.T[rows]                                           # [M, T] gathered
    # tile layout [s, g, p, mi, t]: each (s, g) block is 1 MiB contiguous
    xT = np.ascontiguousarray(
        xg.reshape(MT // 4, 4, P, ST, F).transpose(3, 0, 2, 1, 4)
    )
    cosR = np.cos(theta_R.astype(np.float64)).astype(np.float32)
    sinR = np.sin(theta_R.astype(np.float64)).astype(np.float32)

    in_maps = []
    orders = []
    for cidx in range(NCORES):
        pc = pairs_L[cidx * PPC : (cidx + 1) * PPC]
        n_order = np.concatenate([pc[:, 0], pc[:, 1]])       # [512]
        orders.append(n_order)
        Wt = np.ascontiguousarray(W[n_order][:, m_perm].T)   # [M, 512]
        thL = theta_L[cidx * PPC : (cidx + 1) * PPC].astype(np.float64)
        sfac = (base_row_norms[n_order].astype(np.float64)
                * np.exp(ecd_log_mag[n_order].astype(np.float64)))
        in_maps.append(
            dict(
                xT=xT,
                Wt=Wt,
                cosR=cosR,
                sinR=sinR,
                cosL=np.cos(thL).astype(np.float32),
                sinL=np.sin(thL).astype(np.float32),
                sfac=sfac.astype(np.float32),
                biasv=bias[n_order].astype(np.float32),
            )
        )

    nc = _get_nc()
    run_kwargs = {}
    td = os.environ.get("BASS_TMPDIR")
    if td:
        os.makedirs(td, exist_ok=True)
        run_kwargs["tmpdir"] = td
    res = run_bass_kernel_spmd(nc, in_maps, core_ids=list(range(NCORES)), **run_kwargs)
    global LAST_RESULTS
    LAST_RESULTS = res

    out = np.empty((T, N), dtype=np.float32)
    for cidx in range(NCORES):
        ot = res.results[cidx]["outT"]  # [ST, NT, P, F]
        outT = ot.transpose(1, 2, 0, 3).reshape(NC_ROWS, T)
        out[:, orders[cidx]] = outT.T
    return out
